# revision 3
# baseline (speedup 1.0000x reference)
"""Trainium2 Bass/Tile kernel for the gnn_message_passing problem.

Math (per batch element b, with x = ftr[b] viewed as [C, HW]):
    avg[c] = mean_n x[c,n];  mx[c] = max_n x[c,n]
    cw     = sigmoid(relu(Wa @ avg) + relu(Wm @ mx))              [M]
    k      = relu(Wk @ x + bk)                                    [M, HW]
    kq     = cw[:,None] * k
    S      = sigmoid(kq^T k)                                      [HW, HW]  (never materialized in DRAM)
    d      = (S @ 1)^(-1/2)                                       [HW]
    kd     = k * d[None,:]
    A      = kd @ x^T   (contract HW)                             [M, C]
    mid    = cw[:,None] * A
    LX^T   = x - mid^T' ... computed as  x - (mid as [M,C])^T-contraction with kd:
             LXT[c,n] = x[c,n] - sum_m mid[m,c] kd[m,n]
    out    = x + gcn_w^T @ LXT                                    [C, HW]

Sharding: data-parallel over batch B=8 across 8 cores (1 image per core),
weights replicated. No collectives. Each core computes its own [C, HW] slab.
"""

import numpy as np
from contextlib import ExitStack

import concourse.bass as bass
import concourse.mybir as mybir
import concourse.tile as tile
from concourse import bacc
from concourse.bass_utils import run_bass_kernel_spmd
from concourse.masks import make_identity

F32 = mybir.dt.float32
AF = mybir.ActivationFunctionType
AX = mybir.AxisListType

B, C, H, W = 8, 256, 48, 48
HW = H * W            # 2304
M = 128               # C // 2
P = 128               # partitions
CT = C // P           # 2 c-tiles
NT = HW // P          # 18 n-tiles
N_CORES = 8

S_CHUNK = 1152        # psum chunk of the score matrix (3 banks)
S_HALVES = HW // S_CHUNK  # 2


def _chunks(total, step):
    out = []
    off = 0
    while off < total:
        sz = min(step, total - off)
        out.append((off, sz))
        off += sz
    return out


def build_program(reps=1):
    nc = bacc.Bacc("TRN2", target_bir_lowering=False, debug=False)

    ftr = nc.declare_dram_parameter("ftr", [C, HW], F32, isOutput=False)
    convw = nc.declare_dram_parameter("convw", [M, C], F32, isOutput=False)
    convb = nc.declare_dram_parameter("convb", [M, 1], F32, isOutput=False)
    avgw = nc.declare_dram_parameter("avgw", [M, C], F32, isOutput=False)
    maxw = nc.declare_dram_parameter("maxw", [M, C], F32, isOutput=False)
    gcnw = nc.declare_dram_parameter("gcnw", [C, C], F32, isOutput=False)
    out = nc.declare_dram_parameter("out", [C, HW], F32, isOutput=True)

    with tile.TileContext(nc) as tc:
        for _ in range(reps):
            with ExitStack() as ctx:
                _body(ctx, tc, ftr, convw, convb, avgw, maxw, gcnw, out)
    nc.compile()
    return nc


def _body(ctx, tc, ftr, convw, convb, avgw, maxw, gcnw, out):
    nc = tc.nc

    sb = ctx.enter_context(tc.tile_pool(name="sb", bufs=1))
    scr = ctx.enter_context(tc.tile_pool(name="scr", bufs=2))
    # PSUM: "mm" pool (1 bank x 2) lives for the whole kernel; the big S pool
    # (3 banks x 2) is scoped to the score phase so the tail can reuse banks.
    mmp = ctx.enter_context(tc.tile_pool(name="mmp", bufs=2, space="PSUM"))

    # ---- persistent SBUF tiles ----
    x_sb = sb.tile([P, CT, HW], F32, tag="x")
    xT_sb = sb.tile([P, NT, C], F32, tag="xT")
    k_sb = sb.tile([P, HW], F32, tag="k")
    kq_sb = sb.tile([P, HW], F32, tag="kq")
    kT_sb = sb.tile([P, NT, M], F32, tag="kT")
    kdT_sb = sb.tile([P, NT, M], F32, tag="kdT")
    kd_sb = sb.tile([P, HW], F32, tag="kd")
    lxT_sb = sb.tile([P, CT, HW], F32, tag="lxT")
    yT_sb = sb.tile([P, CT, HW], F32, tag="yT")

    convw_sb = sb.tile([P, C], F32, tag="convw")
    convwT_sb = sb.tile([P, CT, M], F32, tag="convwT")
    convb_sb = sb.tile([P, 1], F32, tag="convb")
    avgw_sb = sb.tile([P, C], F32, tag="avgw")
    avgwT_sb = sb.tile([P, CT, M], F32, tag="avgwT")
    maxw_sb = sb.tile([P, C], F32, tag="maxw")
    maxwT_sb = sb.tile([P, CT, M], F32, tag="maxwT")
    g_sb = sb.tile([P, CT, C], F32, tag="g")
    ident = sb.tile([P, P], F32, tag="ident")

    avg_sb = sb.tile([P, CT, 1], F32, tag="avg")
    mx_sb = sb.tile([P, CT, 1], F32, tag="mx")
    ra_sb = sb.tile([P, 1], F32, tag="ra")
    rm_sb = sb.tile([P, 1], F32, tag="rm")
    cwin_sb = sb.tile([P, 1], F32, tag="cwin")
    cw_sb = sb.tile([P, 1], F32, tag="cw")
    dparts_sb = sb.tile([P, NT, S_HALVES], F32, tag="dparts")
    dsum_sb = sb.tile([P, NT, 1], F32, tag="dsum")
    dinv_sb = sb.tile([P, NT, 1], F32, tag="dinv")
    d_sb = sb.tile([P, NT, 1], F32, tag="d")
    mid_sb = sb.tile([P, C], F32, tag="mid")

    # ---- input DMAs ----
    for ci in range(CT):
        nc.sync.dma_start(out=x_sb[:, ci, :], in_=ftr[ci * P:(ci + 1) * P, :])
    nc.sync.dma_start(out=convw_sb, in_=convw[:, :])
    nc.sync.dma_start(out=convb_sb, in_=convb[:, :])
    nc.sync.dma_start(out=avgw_sb, in_=avgw[:, :])
    nc.sync.dma_start(out=maxw_sb, in_=maxw[:, :])
    for t in range(CT):
        nc.sync.dma_start(out=g_sb[:, t, :], in_=gcnw[t * P:(t + 1) * P, :])

    make_identity(nc, ident)

    # ---- weight transposes: w[M, C] -> wT[c-tile][P, M] ----
    for w_sb, wT in ((convw_sb, convwT_sb), (avgw_sb, avgwT_sb), (maxw_sb, maxwT_sb)):
        ps = mmp.tile([P, 512], F32, tag="mm")
        for ci in range(CT):
            nc.tensor.transpose(ps[:, ci * P:(ci + 1) * P], w_sb[:, ci * P:(ci + 1) * P], ident)
        nc.vector.tensor_copy(wT[:, :, :], ps[:, :C])

    # ---- pooled stats (sum and max over free axis) ----
    for ci in range(CT):
        nc.vector.reduce_sum(out=avg_sb[:, ci, :], in_=x_sb[:, ci, :], axis=AX.X)
        nc.vector.reduce_max(out=mx_sb[:, ci, :], in_=x_sb[:, ci, :], axis=AX.X)

    # ---- k = relu(Wk @ x + b) ----
    for off, sz in _chunks(HW, 512):
        kps = mmp.tile([P, 512], F32, tag="mm")
        for ci in range(CT):
            nc.tensor.matmul(kps[:, :sz], lhsT=convwT_sb[:, ci, :],
                             rhs=x_sb[:, ci, off:off + sz],
                             start=(ci == 0), stop=(ci == CT - 1))
        nc.scalar.activation(out=k_sb[:, off:off + sz], in_=kps[:, :sz],
                             func=AF.Relu, bias=convb_sb[:, :])

    # ---- channel attention cw ----
    aps = mmp.tile([P, 512], F32, tag="mm")
    for ci in range(CT):
        nc.tensor.matmul(aps[:, 0:1], lhsT=avgwT_sb[:, ci, :], rhs=avg_sb[:, ci, :],
                         start=(ci == 0), stop=(ci == CT - 1))
    nc.scalar.activation(out=ra_sb, in_=aps[:, 0:1], func=AF.Relu, scale=1.0 / HW)
    mps = mmp.tile([P, 512], F32, tag="mm")
    for ci in range(CT):
        nc.tensor.matmul(mps[:, 0:1], lhsT=maxwT_sb[:, ci, :], rhs=mx_sb[:, ci, :],
                         start=(ci == 0), stop=(ci == CT - 1))
    nc.scalar.activation(out=rm_sb, in_=mps[:, 0:1], func=AF.Relu)
    nc.vector.tensor_add(cwin_sb, ra_sb, rm_sb)
    nc.scalar.activation(out=cw_sb, in_=cwin_sb, func=AF.Sigmoid)

    # ---- kq = cw * k ----
    nc.vector.tensor_scalar_mul(kq_sb[:, :], k_sb[:, :], cw_sb[:, :])

    # ---- transposes of x and k (overlap with score phase on PE/DVE) ----
    # x: 36 [128,128] tiles -> xT_sb[P, NT, C]; batch 2 n-tiles (4 transposes) per bank
    for j0 in range(0, NT, 2):
        tp = mmp.tile([P, 512], F32, tag="mm")
        for dj in range(2):
            j = j0 + dj
            for ci in range(CT):
                nc.tensor.transpose(tp[:, dj * C + ci * P: dj * C + (ci + 1) * P],
                                    x_sb[:, ci, j * P:(j + 1) * P], ident)
        nc.vector.tensor_copy(xT_sb[:, j0:j0 + 2, :], tp[:, :])
    # k: 18 tiles -> kT_sb[P, NT, M]; batch 4 per bank
    for j0 in range(0, NT, 4):
        nj = min(4, NT - j0)
        tp = mmp.tile([P, 512], F32, tag="mm")
        for dj in range(nj):
            j = j0 + dj
            nc.tensor.transpose(tp[:, dj * P:(dj + 1) * P],
                                k_sb[:, j * P:(j + 1) * P], ident)
        nc.vector.tensor_copy(kT_sb[:, j0:j0 + nj, :], tp[:, :nj * P])

    # ---- score phase: S = sigmoid(kq^T k), d row-sums via ACT accumulate ----
    with tc.tile_pool(name="sps", bufs=2, space="PSUM") as sps:
        for nt in range(NT):
            lhsT = kq_sb[:, nt * P:(nt + 1) * P]
            for h in range(S_HALVES):
                sp = sps.tile([P, S_CHUNK], F32, tag="s")
                base = h * S_CHUNK
                for off, sz in _chunks(S_CHUNK, 512):
                    nc.tensor.matmul(sp[:, off:off + sz], lhsT=lhsT,
                                     rhs=k_sb[:, base + off:base + off + sz],
                                     start=True, stop=True)
                sig = scr.tile([P, S_CHUNK], F32, tag="sig")
                nc.scalar.activation(out=sig, in_=sp, func=AF.Sigmoid,
                                     accum_out=dparts_sb[:, nt, h:h + 1])

    # ---- d = rowsum^-1/2 ----
    nc.vector.reduce_sum(out=dsum_sb[:, :, :], in_=dparts_sb[:, :, :], axis=AX.X)
    nc.vector.reciprocal(out=dinv_sb[:, :, :], in_=dsum_sb[:, :, :])
    nc.scalar.activation(out=d_sb[:, :, :], in_=dinv_sb[:, :, :], func=AF.Sqrt)

    # ---- kdT = kT * d (per-partition scalar), kd = transpose(kdT) ----
    for nt in range(NT):
        nc.vector.tensor_scalar_mul(kdT_sb[:, nt, :], kT_sb[:, nt, :], d_sb[:, nt, :])
    for j0 in range(0, NT, 4):
        nj = min(4, NT - j0)
        tp = mmp.tile([P, 512], F32, tag="mm")
        for dj in range(nj):
            j = j0 + dj
            nc.tensor.transpose(tp[:, dj * P:(dj + 1) * P], kdT_sb[:, j, :], ident)
        nc.vector.tensor_copy(kd_sb[:, j0 * P:(j0 + nj) * P], tp[:, :nj * P])

    # tail psum pool (S pool is released now)
    with tc.tile_pool(name="tailp", bufs=4, space="PSUM") as tailp:
        # ---- A = kd @ x^T  (contract HW);  mid = cw * A ----
        a_ps = tailp.tile([P, 512], F32, tag="t")
        for nt in range(NT):
            nc.tensor.matmul(a_ps[:, :C], lhsT=kdT_sb[:, nt, :], rhs=xT_sb[:, nt, :],
                             start=(nt == 0), stop=(nt == NT - 1))
        nc.vector.tensor_scalar_mul(mid_sb[:, :], a_ps[:, :C], cw_sb[:, :])

        # ---- LXT = x - mid^T-contraction kd ----
        for ci in range(CT):
            for off, sz in _chunks(HW, 512):
                tp = tailp.tile([P, 512], F32, tag="t")
                nc.tensor.matmul(tp[:, :sz], lhsT=mid_sb[:, ci * P:(ci + 1) * P],
                                 rhs=kd_sb[:, off:off + sz], start=True, stop=True)
                nc.vector.tensor_sub(lxT_sb[:, ci, off:off + sz],
                                     x_sb[:, ci, off:off + sz], tp[:, :sz])

        # ---- out = x + gcn_w^T @ LXT ----
        for ci in range(CT):
            for off, sz in _chunks(HW, 512):
                yp = tailp.tile([P, 512], F32, tag="t")
                for t in range(CT):
                    nc.tensor.matmul(yp[:, :sz], lhsT=g_sb[:, t, ci * P:(ci + 1) * P],
                                     rhs=lxT_sb[:, t, off:off + sz],
                                     start=(t == 0), stop=(t == CT - 1))
                nc.vector.tensor_add(yT_sb[:, ci, off:off + sz],
                                     x_sb[:, ci, off:off + sz], yp[:, :sz])
                nc.sync.dma_start(out=out[ci * P:(ci + 1) * P, off:off + sz],
                                  in_=yT_sb[:, ci, off:off + sz])


_PROGRAM = None


def _get_program():
    global _PROGRAM
    if _PROGRAM is None:
        _PROGRAM = build_program()
    return _PROGRAM


def _in_maps(ftr, conv_k_w, conv_k_b, avg_fc_w, max_fc_w, gcn_w):
    wmaps = {
        "convw": np.ascontiguousarray(conv_k_w, dtype=np.float32),
        "convb": np.ascontiguousarray(np.asarray(conv_k_b, dtype=np.float32).reshape(M, 1)),
        "avgw": np.ascontiguousarray(avg_fc_w, dtype=np.float32),
        "maxw": np.ascontiguousarray(max_fc_w, dtype=np.float32),
        "gcnw": np.ascontiguousarray(gcn_w, dtype=np.float32),
    }
    return [
        {"ftr": np.ascontiguousarray(np.asarray(ftr[b], dtype=np.float32).reshape(C, HW)), **wmaps}
        for b in range(B)
    ]


def kernel(ftr, conv_k_w, conv_k_b, avg_fc_w, max_fc_w, gcn_w):
    nc = _get_program()
    in_maps = _in_maps(ftr, conv_k_w, conv_k_b, avg_fc_w, max_fc_w, gcn_w)
    res = run_bass_kernel_spmd(nc, in_maps, core_ids=list(range(N_CORES)))
    outs = [np.asarray(res.results[b]["out"]).reshape(C, H, W) for b in range(B)]
    return np.stack(outs, axis=0).astype(np.float32)


# revision 6
# speedup vs baseline: 3.3704x; 3.3704x over previous
"""Trainium2 Bass/Tile kernel for the gnn_message_passing problem.

Math (per batch element b, with x = ftr[b] viewed as [C, HW]):
    avg[c] = mean_n x[c,n];  mx[c] = max_n x[c,n]
    cw     = sigmoid(relu(Wa @ avg) + relu(Wm @ mx))              [M]
    k      = relu(Wk @ x + bk)                                    [M, HW]
    kq     = cw[:,None] * k
    S      = sigmoid(kq^T k)                                      [HW, HW]  (never materialized in DRAM)
    d      = (S @ 1)^(-1/2)                                       [HW]
    kd     = k * d[None,:]
    A      = kd @ x^T   (contract HW)                             [M, C]
    mid    = cw[:,None] * A
    LX^T   = x - mid^T' ... computed as  x - (mid as [M,C])^T-contraction with kd:
             LXT[c,n] = x[c,n] - sum_m mid[m,c] kd[m,n]
    out    = x + gcn_w^T @ LXT                                    [C, HW]

Sharding: data-parallel over batch B=8 across 8 cores (1 image per core),
weights replicated. No collectives. Each core computes its own [C, HW] slab.
"""

import numpy as np
from contextlib import ExitStack

import concourse.bass as bass
import concourse.mybir as mybir
import concourse.tile as tile
from concourse import bacc
from concourse.bass_utils import run_bass_kernel_spmd
from concourse.masks import make_identity

F32 = mybir.dt.float32
F32R = mybir.dt.float32r  # fp32 bits, 4x-faster PE streaming mode
AF = mybir.ActivationFunctionType
AX = mybir.AxisListType


def _r(ap):
    return ap.bitcast(F32R)

B, C, H, W = 8, 256, 48, 48
HW = H * W            # 2304
M = 128               # C // 2
P = 128               # partitions
CT = C // P           # 2 c-tiles
NT = HW // P          # 18 n-tiles
N_CORES = 8

S_CHUNK = 1152        # psum chunk of the score matrix (3 banks)
S_HALVES = HW // S_CHUNK  # 2


def _chunks(total, step):
    out = []
    off = 0
    while off < total:
        sz = min(step, total - off)
        out.append((off, sz))
        off += sz
    return out


def build_program(reps=1):
    nc = bacc.Bacc("TRN2", target_bir_lowering=False, debug=False)

    ftr = nc.declare_dram_parameter("ftr", [C, HW], F32, isOutput=False)
    convw = nc.declare_dram_parameter("convw", [M, C], F32, isOutput=False)
    convb = nc.declare_dram_parameter("convb", [M, 1], F32, isOutput=False)
    avgw = nc.declare_dram_parameter("avgw", [M, C], F32, isOutput=False)
    maxw = nc.declare_dram_parameter("maxw", [M, C], F32, isOutput=False)
    gcnw = nc.declare_dram_parameter("gcnw", [C, C], F32, isOutput=False)
    out = nc.declare_dram_parameter("out", [C, HW], F32, isOutput=True)

    with tile.TileContext(nc) as tc:
        for _ in range(reps):
            with ExitStack() as ctx:
                _body(ctx, tc, ftr, convw, convb, avgw, maxw, gcnw, out)
    nc.compile()
    return nc


def _body(ctx, tc, ftr, convw, convb, avgw, maxw, gcnw, out):
    nc = tc.nc

    sb = ctx.enter_context(tc.tile_pool(name="sb", bufs=1))
    scr = ctx.enter_context(tc.tile_pool(name="scr", bufs=2))
    # PSUM: "mm" pool (1 bank x 2) lives for the whole kernel; the big S pool
    # (3 banks x 2) is scoped to the score phase so the tail can reuse banks.
    mmp = ctx.enter_context(tc.tile_pool(name="mmp", bufs=2, space="PSUM"))

    # ---- persistent SBUF tiles ----
    x_sb = sb.tile([P, CT, HW], F32, tag="x")
    xT_sb = sb.tile([P, NT, C], F32R, tag="xT")
    k_sb = sb.tile([P, HW], F32R, tag="k")
    kq_sb = sb.tile([P, HW], F32R, tag="kq")
    kT_sb = sb.tile([P, NT, M], F32R, tag="kT")
    kdT_sb = sb.tile([P, NT, M], F32R, tag="kdT")
    kd_sb = sb.tile([P, HW], F32R, tag="kd")
    lxT_sb = sb.tile([P, CT, HW], F32R, tag="lxT")
    yT_sb = sb.tile([P, CT, HW], F32, tag="yT")

    convw_sb = sb.tile([P, C], F32, tag="convw")
    convwT_sb = sb.tile([P, CT, M], F32R, tag="convwT")
    convb_sb = sb.tile([P, 1], F32, tag="convb")
    avgw_sb = sb.tile([P, C], F32, tag="avgw")
    avgwT_sb = sb.tile([P, CT, M], F32, tag="avgwT")
    maxw_sb = sb.tile([P, C], F32, tag="maxw")
    maxwT_sb = sb.tile([P, CT, M], F32, tag="maxwT")
    g_sb = sb.tile([P, CT, C], F32, tag="g")
    xr_sb = sb.tile([P, CT, HW], F32R, tag="xr")
    gr_sb = sb.tile([P, CT, C], F32R, tag="gr")
    ident = sb.tile([P, P], F32, tag="ident")

    avg_sb = sb.tile([P, CT, 1], F32, tag="avg")
    mx_sb = sb.tile([P, CT, 1], F32, tag="mx")
    ra_sb = sb.tile([P, 1], F32, tag="ra")
    rm_sb = sb.tile([P, 1], F32, tag="rm")
    cwin_sb = sb.tile([P, 1], F32, tag="cwin")
    cw_sb = sb.tile([P, 1], F32, tag="cw")
    dparts_sb = sb.tile([P, NT, S_HALVES], F32, tag="dparts")
    dsum_sb = sb.tile([P, NT, 1], F32, tag="dsum")
    dinv_sb = sb.tile([P, NT, 1], F32, tag="dinv")
    d_sb = sb.tile([P, NT, 1], F32, tag="d")
    mid_sb = sb.tile([P, C], F32R, tag="mid")

    # ---- input DMAs ----
    for ci in range(CT):
        nc.sync.dma_start(out=x_sb[:, ci, :], in_=ftr[ci * P:(ci + 1) * P, :])
    nc.sync.dma_start(out=convw_sb, in_=convw[:, :])
    nc.sync.dma_start(out=convb_sb, in_=convb[:, :])
    nc.sync.dma_start(out=avgw_sb, in_=avgw[:, :])
    nc.sync.dma_start(out=maxw_sb, in_=maxw[:, :])
    for t in range(CT):
        nc.sync.dma_start(out=g_sb[:, t, :], in_=gcnw[t * P:(t + 1) * P, :])

    make_identity(nc, ident)

    # fp32r rounding copies of DMA-landed matmul operands
    for ci in range(CT):
        nc.vector.tensor_copy(xr_sb[:, ci, :], x_sb[:, ci, :])
    nc.vector.tensor_copy(gr_sb[:, :, :], g_sb[:, :, :])

    # ---- weight transposes: w[M, C] -> wT[c-tile][P, M] ----
    for w_sb, wT in ((convw_sb, convwT_sb), (avgw_sb, avgwT_sb), (maxw_sb, maxwT_sb)):
        ps = mmp.tile([P, 512], F32, tag="mm")
        for ci in range(CT):
            nc.tensor.transpose(ps[:, ci * P:(ci + 1) * P], w_sb[:, ci * P:(ci + 1) * P], ident)
        nc.vector.tensor_copy(wT[:, :, :], ps[:, :C])

    # ---- pooled stats (sum and max over free axis) ----
    for ci in range(CT):
        nc.vector.reduce_sum(out=avg_sb[:, ci, :], in_=x_sb[:, ci, :], axis=AX.X)
        nc.vector.reduce_max(out=mx_sb[:, ci, :], in_=x_sb[:, ci, :], axis=AX.X)

    # ---- k = relu(Wk @ x + b) ----
    for off, sz in _chunks(HW, 512):
        kps = mmp.tile([P, 512], F32, tag="mm")
        for ci in range(CT):
            nc.tensor.matmul(kps[:, :sz], lhsT=convwT_sb[:, ci, :],
                             rhs=xr_sb[:, ci, off:off + sz],
                             start=(ci == 0), stop=(ci == CT - 1))
        nc.scalar.activation(out=k_sb[:, off:off + sz], in_=kps[:, :sz],
                             func=AF.Relu, bias=convb_sb[:, :])

    # ---- channel attention cw ----
    aps = mmp.tile([P, 512], F32, tag="mm")
    for ci in range(CT):
        nc.tensor.matmul(aps[:, 0:1], lhsT=avgwT_sb[:, ci, :], rhs=avg_sb[:, ci, :],
                         start=(ci == 0), stop=(ci == CT - 1))
    nc.scalar.activation(out=ra_sb, in_=aps[:, 0:1], func=AF.Relu, scale=1.0 / HW)
    mps = mmp.tile([P, 512], F32, tag="mm")
    for ci in range(CT):
        nc.tensor.matmul(mps[:, 0:1], lhsT=maxwT_sb[:, ci, :], rhs=mx_sb[:, ci, :],
                         start=(ci == 0), stop=(ci == CT - 1))
    nc.scalar.activation(out=rm_sb, in_=mps[:, 0:1], func=AF.Relu)
    nc.vector.tensor_add(cwin_sb, ra_sb, rm_sb)
    nc.scalar.activation(out=cw_sb, in_=cwin_sb, func=AF.Sigmoid)

    # ---- kq = cw * k ----
    nc.vector.tensor_scalar_mul(kq_sb[:, :], k_sb[:, :], cw_sb[:, :])

    # ---- transposes of x and k (overlap with score phase on PE/DVE) ----
    # x: 36 [128,128] tiles -> xT_sb[P, NT, C]; batch 2 n-tiles (4 transposes) per bank
    for j0 in range(0, NT, 2):
        tp = mmp.tile([P, 512], F32, tag="mm")
        for dj in range(2):
            j = j0 + dj
            for ci in range(CT):
                nc.tensor.transpose(tp[:, dj * C + ci * P: dj * C + (ci + 1) * P],
                                    x_sb[:, ci, j * P:(j + 1) * P], ident)
        nc.vector.tensor_copy(xT_sb[:, j0:j0 + 2, :], tp[:, :])
    # k: 18 tiles -> kT_sb[P, NT, M]; batch 4 per bank
    for j0 in range(0, NT, 4):
        nj = min(4, NT - j0)
        tp = mmp.tile([P, 512], F32, tag="mm")
        for dj in range(nj):
            j = j0 + dj
            nc.tensor.transpose(tp[:, dj * P:(dj + 1) * P],
                                k_sb[:, j * P:(j + 1) * P].bitcast(F32), ident)
        nc.vector.tensor_copy(kT_sb[:, j0:j0 + nj, :], tp[:, :nj * P])

    # ---- score phase: S = sigmoid(kq^T k), d row-sums via ACT accumulate ----
    with tc.tile_pool(name="sps", bufs=2, space="PSUM") as sps:
        for nt in range(NT):
            lhsT = kq_sb[:, nt * P:(nt + 1) * P]
            for h in range(S_HALVES):
                sp = sps.tile([P, S_CHUNK], F32, tag="s")
                base = h * S_CHUNK
                for off, sz in _chunks(S_CHUNK, 512):
                    nc.tensor.matmul(sp[:, off:off + sz], lhsT=lhsT,
                                     rhs=k_sb[:, base + off:base + off + sz],
                                     start=True, stop=True)
                sig = scr.tile([P, S_CHUNK], F32, tag="sig")
                nc.scalar.activation(out=sig, in_=sp, func=AF.Sigmoid,
                                     accum_out=dparts_sb[:, nt, h:h + 1])

    # ---- d = rowsum^-1/2 ----
    nc.vector.reduce_sum(out=dsum_sb[:, :, :], in_=dparts_sb[:, :, :], axis=AX.X)
    nc.vector.reciprocal(out=dinv_sb[:, :, :], in_=dsum_sb[:, :, :])
    nc.scalar.activation(out=d_sb[:, :, :], in_=dinv_sb[:, :, :], func=AF.Sqrt)

    # ---- kdT = kT * d (per-partition scalar), kd = transpose(kdT) ----
    for nt in range(NT):
        nc.vector.tensor_scalar_mul(kdT_sb[:, nt, :], kT_sb[:, nt, :], d_sb[:, nt, :])
    for j0 in range(0, NT, 4):
        nj = min(4, NT - j0)
        tp = mmp.tile([P, 512], F32, tag="mm")
        for dj in range(nj):
            j = j0 + dj
            nc.tensor.transpose(tp[:, dj * P:(dj + 1) * P],
                                kdT_sb[:, j, :].bitcast(F32), ident)
        nc.vector.tensor_copy(kd_sb[:, j0 * P:(j0 + nj) * P], tp[:, :nj * P])

    # tail psum pool (S pool is released now)
    with tc.tile_pool(name="tailp", bufs=4, space="PSUM") as tailp:
        # ---- A = kd @ x^T  (contract HW);  mid = cw * A ----
        a_ps = tailp.tile([P, 512], F32, tag="t")
        for nt in range(NT):
            nc.tensor.matmul(a_ps[:, :C], lhsT=kdT_sb[:, nt, :], rhs=xT_sb[:, nt, :],
                             start=(nt == 0), stop=(nt == NT - 1))
        nc.vector.tensor_scalar_mul(mid_sb[:, :], a_ps[:, :C], cw_sb[:, :])

        # ---- LXT = x - mid^T-contraction kd ----
        for ci in range(CT):
            for off, sz in _chunks(HW, 512):
                tp = tailp.tile([P, 512], F32, tag="t")
                nc.tensor.matmul(tp[:, :sz], lhsT=mid_sb[:, ci * P:(ci + 1) * P],
                                 rhs=kd_sb[:, off:off + sz], start=True, stop=True)
                nc.vector.tensor_sub(lxT_sb[:, ci, off:off + sz],
                                     x_sb[:, ci, off:off + sz], tp[:, :sz])

        # ---- out = x + gcn_w^T @ LXT ----
        for ci in range(CT):
            for off, sz in _chunks(HW, 512):
                yp = tailp.tile([P, 512], F32, tag="t")
                for t in range(CT):
                    nc.tensor.matmul(yp[:, :sz], lhsT=gr_sb[:, t, ci * P:(ci + 1) * P],
                                     rhs=lxT_sb[:, t, off:off + sz],
                                     start=(t == 0), stop=(t == CT - 1))
                nc.vector.tensor_add(yT_sb[:, ci, off:off + sz],
                                     x_sb[:, ci, off:off + sz], yp[:, :sz])
                nc.sync.dma_start(out=out[ci * P:(ci + 1) * P, off:off + sz],
                                  in_=yT_sb[:, ci, off:off + sz])


_PROGRAM = None


def _get_program():
    global _PROGRAM
    if _PROGRAM is None:
        _PROGRAM = build_program()
    return _PROGRAM


def _in_maps(ftr, conv_k_w, conv_k_b, avg_fc_w, max_fc_w, gcn_w):
    wmaps = {
        "convw": np.ascontiguousarray(conv_k_w, dtype=np.float32),
        "convb": np.ascontiguousarray(np.asarray(conv_k_b, dtype=np.float32).reshape(M, 1)),
        "avgw": np.ascontiguousarray(avg_fc_w, dtype=np.float32),
        "maxw": np.ascontiguousarray(max_fc_w, dtype=np.float32),
        "gcnw": np.ascontiguousarray(gcn_w, dtype=np.float32),
    }
    return [
        {"ftr": np.ascontiguousarray(np.asarray(ftr[b], dtype=np.float32).reshape(C, HW)), **wmaps}
        for b in range(B)
    ]


def kernel(ftr, conv_k_w, conv_k_b, avg_fc_w, max_fc_w, gcn_w):
    nc = _get_program()
    in_maps = _in_maps(ftr, conv_k_w, conv_k_b, avg_fc_w, max_fc_w, gcn_w)
    res = run_bass_kernel_spmd(nc, in_maps, core_ids=list(range(N_CORES)))
    outs = [np.asarray(res.results[b]["out"]).reshape(C, H, W) for b in range(B)]
    return np.stack(outs, axis=0).astype(np.float32)
